# revision 25
# baseline (speedup 1.0000x reference)
"""BigBird attention kernel for 8 Trainium2 NeuronCores.

Sharding: data-parallel over batch (2) x tensor-parallel over heads (4 groups
of 4 heads) = 8 cores. Each core computes q/k/v projections for its head
slice, block-sparse masked attention over 128x128 supertiles derived from the
runtime mask, and a partial output projection with its Wo row-slice. The host
sums the 4 partial bf16 outputs per batch in fp32.

All matmuls run in bf16 (1 cycle/row). Scores are computed transposed
(S^T[k, q]) so the post-softmax P^T feeds attn@V directly as the stationary
operand - no per-supertile transpose matmuls. Head pairs within a 128-row
chunk run as concurrent row-tiled matmuls (tile_position (0,0)/(64,0)). The
softmax row sum rides as a ones-column appended to V; division happens on the
[q, 65] attention output (q = partition dim, native per-partition scale),
then one 128x128 transpose matmul per output chunk feeds the Wo projection.
"""

import sys

for _p in ("/opt/trn_rl_repo", "/opt/trn_rl_repo/concourse"):
    if _p not in sys.path:
        sys.path.insert(0, _p)

import numpy as np

import concourse.bacc as bacc
import concourse.bass as bass
import concourse.mybir as mybir
import concourse.tile as tile
from concourse import bass_utils

F32 = mybir.dt.float32
BF16 = mybir.dt.bfloat16

B, S, D, H = 2, 2048, 1024, 16
HD = D // H          # 64
SCALE = 1.0 / float(np.sqrt(HD))
NCORES = 8
HG = 4               # head groups (tensor-parallel)
HPC = H // HG        # heads per core = 4
DC = HPC * HD        # channels per core = 256
QT = 128             # supertile edge
NQ = S // QT         # 16
VW = HD + 1          # v columns per head incl. ones column (65)
VWP = HD + 2         # padded to 66 so bf16 head blocks stay 4B-aligned
KC = D // 128        # 8 contraction chunks
CC = DC // 128       # 2 channel chunks (2 heads each)


def _mask_pattern(mask):
    """Active 128x128 supertiles per q-tile from the runtime mask."""
    sup = mask.reshape(NQ, QT, NQ, QT).any(axis=(1, 3))  # [16,16]
    kts = [np.nonzero(sup[qi])[0].tolist() for qi in range(NQ)]
    cnts = [len(k) for k in kts]
    maxw = max(max(cnts), 1) * QT
    return kts, cnts, maxw


def _build_nc(kts, cnts, maxw):
    nc = bacc.Bacc("TRN2", target_bir_lowering=False, debug=False)

    sumw = sum(cnts) * QT
    moff = [0]
    for qi in range(NQ):
        moff.append(moff[-1] + cnts[qi] * QT)

    xT_d = nc.dram_tensor("xT", [128, KC, S], BF16, kind="ExternalInput")
    wq_d = nc.dram_tensor("wq", [128, KC, DC], BF16, kind="ExternalInput")
    wk_d = nc.dram_tensor("wk", [128, KC, DC], BF16, kind="ExternalInput")
    wv_d = nc.dram_tensor("wv", [128, KC, HPC * VWP], BF16, kind="ExternalInput")
    wo_d = nc.dram_tensor("wo", [128, CC, D], BF16, kind="ExternalInput")
    cos_d = nc.dram_tensor("cosT", [128, S], BF16, kind="ExternalInput")
    sin_d = nc.dram_tensor("sinT", [128, S], BF16, kind="ExternalInput")
    rt_d = nc.dram_tensor("rT", [128, 128], BF16, kind="ExternalInput")
    id_d = nc.dram_tensor("ident", [128, 128], BF16, kind="ExternalInput")
    mk_d = nc.dram_tensor("maskT", [128, sumw], BF16, kind="ExternalInput")
    out_d = nc.dram_tensor("out", [S, D], BF16, kind="ExternalOutput")

    with tile.TileContext(nc) as tc:
        with (
            tc.tile_pool(name="persist", bufs=1) as pp,
        ):
            # persistent sbuf tensors
            qrT = [pp.tile([128, S], BF16, tag=f"qrT{c}", name=f"qrT{c}") for c in range(CC)]
            krT = [pp.tile([128, S], BF16, tag=f"krT{c}", name=f"krT{c}") for c in range(CC)]
            v_sb = [pp.tile([128, HPC, VWP], BF16, tag=f"v{i}", name=f"v{i}") for i in range(NQ)]
            otT = pp.tile([128, CC, S], BF16, tag="otT", name="otT")
            wo_sb = pp.tile([128, CC, D], BF16, tag="wo", name="wo")
            identb = pp.tile([128, 128], BF16, tag="ident")
            mk_sb = pp.tile([128, sumw], BF16, tag="maskT", name="maskT")

            # ---------------- QKV + RoPE ----------------
            with (
                tc.tile_pool(name="qkv_in", bufs=1) as qp,
                tc.tile_pool(name="qkv_scr", bufs=4) as sp,
                tc.tile_pool(name="qkv_ps", bufs=2, space="PSUM") as psp,
                tc.tile_pool(name="qkv_rot", bufs=2, space="PSUM") as psr,
                tc.tile_pool(name="qkv_psv", bufs=2, space="PSUM") as psv,
            ):
                wq_sb = qp.tile([128, KC, DC], BF16, tag="wq", name="wq")
                wk_sb = qp.tile([128, KC, DC], BF16, tag="wk", name="wk")
                wv_sb = qp.tile([128, KC, HPC * VWP], BF16, tag="wv", name="wv")
                cosT = qp.tile([128, S], BF16, tag="cosT")
                sinT = qp.tile([128, S], BF16, tag="sinT")
                rT = qp.tile([128, 128], BF16, tag="rT")
                xT = qp.tile([128, KC, S], BF16, tag="xT", name="xT")
                # DMA order = dispatch order: first-needed first
                nc.sync.dma_start(wq_sb[:], wq_d[:, :, :])
                nc.sync.dma_start(xT[:, :, 0:512], xT_d[:, :, 0:512])
                nc.sync.dma_start(wk_sb[:], wk_d[:, :, :])
                nc.sync.dma_start(cosT[:], cos_d[:, :])
                nc.sync.dma_start(sinT[:], sin_d[:, :])
                nc.sync.dma_start(rT[:], rt_d[:, :])
                for pc in range(1, 4):
                    fs = slice(pc * 512, (pc + 1) * 512)
                    nc.sync.dma_start(xT[:, :, fs], xT_d[:, :, fs])
                nc.sync.dma_start(wv_sb[:], wv_d[:, :, :])
                nc.sync.dma_start(identb[:], id_d[:, :])
                nc.sync.dma_start(wo_sb[:], wo_d[:, :, :])
                nc.sync.dma_start(mk_sb[:], mk_d[:, :])

                # q^T / k^T with rope applied
                for pc in range(S // 512):
                    fs = slice(pc * 512, (pc + 1) * 512)
                    for cc in range(CC):
                        for w_sb, dstT in (
                            (wq_sb, qrT),
                            (wk_sb, krT),
                        ):
                            ps = psp.tile([128, 512], F32, tag="ps_qk", name="ps_qk")
                            for k in range(KC):
                                nc.tensor.matmul(
                                    ps[:],
                                    w_sb[:, k, cc * 128:(cc + 1) * 128],
                                    xT[:, k, fs],
                                    start=(k == 0),
                                    stop=(k == KC - 1),
                                )
                            raw = sp.tile([128, 512], BF16, tag="raw")
                            nc.scalar.copy(raw[:], ps[:])
                            rot = psr.tile([128, 512], F32, tag="rot")
                            nc.tensor.matmul(
                                rot[:], rT[:], raw[:], start=True, stop=True
                            )
                            u = sp.tile([128, 512], BF16, tag="u")
                            nc.vector.tensor_mul(u[:], rot[:], sinT[:, fs])
                            nc.vector.tensor_mul(dstT[cc][:, fs], raw[:], cosT[:, fs])
                            nc.vector.tensor_add(
                                dstT[cc][:, fs], dstT[cc][:, fs], u[:]
                            )

                # v natural [s, 4*66] with a ones column per head
                for pi in range(NQ):
                    ps_v = psv.tile([128, 512], F32, tag="ps_v")
                    for k in range(KC):
                        nc.tensor.matmul(
                            ps_v[:, 0:HPC * VWP],
                            xT[:, k, pi * 128:(pi + 1) * 128],
                            wv_sb[:, k, :],
                            start=(k == 0),
                            stop=(k == KC - 1),
                        )
                    nc.scalar.copy(v_sb[pi][:, :, :], ps_v[:, 0:HPC * VWP])
                    nc.vector.memset(v_sb[pi][:, :, HD:VW], 1.0)

            # ---------------- attention + output projection ----------------
            with (
                tc.tile_pool(name="at_p", bufs=3) as ep,
                tc.tile_pool(name="at_sc", bufs=2) as scp,
                tc.tile_pool(name="at_ob", bufs=2) as obp,
                tc.tile_pool(name="ps_s", bufs=3, space="PSUM") as pss,
                tc.tile_pool(name="ps_o", bufs=2, space="PSUM") as pso,
                tc.tile_pool(name="ps_t", bufs=1, space="PSUM") as pst,
                tc.tile_pool(name="ps_w", bufs=2, space="PSUM") as psw,
            ):
                for qi in range(NQ):
                    cnt = cnts[qi]
                    w = cnt * QT
                    qs = slice(qi * 128, (qi + 1) * 128)
                    po = pso.tile([128, HPC, 128], F32, tag="po", name="po")
                    # sub-blocks of <=4 supertiles keep scores psum in 1-bank tiles
                    subs = [(s, min(s + 4, cnt)) for s in range(0, cnt, 4)]
                    for cc in range(CC):
                        pT = [
                            ep.tile([128, maxw], BF16, tag=f"pT{hh}", name="pT")
                            for hh in range(2)
                        ]
                        for s0, s1 in subs:
                            # paired heads 2cc/2cc+1: concurrent row-tiled scores
                            ps_pair = [
                                pss.tile([128, 512], F32, tag="ps_sc", name="ps_sc")
                                for _ in range(2)
                            ]
                            for j in range(s0, s1):
                                kt = kts[qi][j]
                                for hh in range(2):
                                    ho = hh * 64
                                    nc.tensor.matmul(
                                        ps_pair[hh][:, (j - s0) * 128:(j - s0 + 1) * 128],
                                        krT[cc][ho:ho + 64, kt * 128:(kt + 1) * 128],
                                        qrT[cc][ho:ho + 64, qs],
                                        start=True,
                                        stop=True,
                                    )
                            for hh in range(2):
                                nc.scalar.activation(
                                    pT[hh][:, s0 * 128:s1 * 128],
                                    ps_pair[hh][:, 0:(s1 - s0) * 128],
                                    mybir.ActivationFunctionType.Exp,
                                    bias=0.0,
                                    scale=SCALE,
                                )
                        for hh in range(2):
                            h = 2 * cc + hh
                            pTm = ep.tile([128, maxw], BF16, tag="pTm", name="pTm")
                            nc.vector.tensor_mul(
                                pTm[:, :w], pT[hh][:, :w],
                                mk_sb[:, moff[qi]:moff[qi] + w],
                            )
                            # attn @ V' (ones column gives softmax row sums)
                            for j, kt in enumerate(kts[qi]):
                                nc.tensor.matmul(
                                    po[:, h:h + 1, 0:VW],
                                    pTm[:, j * 128:(j + 1) * 128],
                                    v_sb[kt][:, h:h + 1, 0:VW],
                                    start=(j == 0),
                                    stop=(j == cnt - 1),
                                )
                    # softmax division (per-partition scale on natural-q layout)
                    r = scp.tile([128, HPC], F32, tag="r", name="r")
                    nc.vector.reciprocal(r[:, :], po[:, :, HD:VW])
                    o_nat = scp.tile([128, DC], BF16, tag="o_nat", name="o_nat")
                    for h in range(HPC):
                        nc.vector.tensor_scalar_mul(
                            o_nat[:, h * HD:(h + 1) * HD],
                            po[:, h:h + 1, 0:HD],
                            r[:, h:h + 1],
                        )
                    # transpose O[q, c] -> otT[c, q]
                    ot_ps = pst.tile([128, 256], F32, tag="ot_ps", name="ot_ps")
                    for c2 in range(CC):
                        nc.tensor.matmul(
                            ot_ps[:, c2 * 128:(c2 + 1) * 128],
                            o_nat[:, c2 * 128:(c2 + 1) * 128],
                            identb[:],
                            start=True,
                            stop=True,
                        )
                    nc.vector.tensor_copy(otT[:, :, qs], ot_ps[:, 0:256])
                    # output projection for this q chunk
                    ob = obp.tile([128, D], BF16, tag="ob", name="ob")
                    for n2 in range(2):
                        pw = psw.tile([128, 512], F32, tag="pw", name="pw")
                        for cc2 in range(CC):
                            nc.tensor.matmul(
                                pw[:],
                                otT[:, cc2, qs],
                                wo_sb[:, cc2, n2 * 512:(n2 + 1) * 512],
                                start=(cc2 == 0),
                                stop=(cc2 == CC - 1),
                            )
                        nc.vector.tensor_copy(
                            ob[:, n2 * 512:(n2 + 1) * 512], pw[:]
                        )
                    nc.sync.dma_start(out_d[qi * 128:(qi + 1) * 128, :], ob[:])

    nc.compile()
    return nc


def _host_inputs(x, freqs_cos, freqs_sin, position_ids, mask01, kts, cnts, maxw,
                 Wq, Wk, Wv, Wo):
    """Per-core input maps."""
    import ml_dtypes
    bf = ml_dtypes.bfloat16

    in_maps = []
    r64 = np.zeros((HD, HD), np.float32)
    for i in range(HD // 2):
        r64[2 * i, 2 * i + 1] = -1.0
        r64[2 * i + 1, 2 * i] = 1.0
    r128 = np.zeros((128, 128), np.float32)
    r128[:64, :64] = r64
    r128[64:, 64:] = r64
    rT = np.ascontiguousarray(r128.T).astype(bf)
    ident = np.eye(128, dtype=np.float32).astype(bf)

    # transposed mask supertiles packed per q-tile: [k_local, off[qi] + j*128 + q_local]
    sumw = sum(cnts) * QT
    maskT = np.zeros((QT, sumw), bf)
    off = 0
    for qi in range(NQ):
        for j, kt in enumerate(kts[qi]):
            maskT[:, off + j * QT:off + (j + 1) * QT] = mask01[
                qi * QT:(qi + 1) * QT, kt * QT:(kt + 1) * QT
            ].T
        off += cnts[qi] * QT

    def perm3(a, inner):
        # [1024, inner] -> [128, 8, inner]
        return np.ascontiguousarray(
            a.reshape(KC, 128, inner).transpose(1, 0, 2)
        ).astype(bf)

    for c in range(NCORES):
        b, g = c // HG, c % HG
        pos = np.clip(position_ids[b].astype(np.int64), 0, freqs_cos.shape[0] - 1)
        cos_g = np.asarray(freqs_cos)[pos]  # [S, 32]
        sin_g = np.asarray(freqs_sin)[pos]
        cosT64 = np.repeat(cos_g.T, 2, axis=0).astype(np.float32)  # [64, S]
        sinT64 = np.repeat(sin_g.T, 2, axis=0).astype(np.float32)
        cs = slice(g * DC, (g + 1) * DC)
        wv_g = np.asarray(Wv)[:, cs].astype(np.float32)  # [D, 256]
        wv260 = np.zeros((D, HPC * VWP), np.float32)
        for h in range(HPC):
            wv260[:, h * VWP:h * VWP + HD] = wv_g[:, h * HD:(h + 1) * HD]
        wo_g = np.asarray(Wo)[cs, :].astype(np.float32)  # [256, 1024]
        wo3 = np.ascontiguousarray(
            wo_g.reshape(CC, 128, D).transpose(1, 0, 2)
        ).astype(bf)
        in_maps.append({
            "xT": perm3(np.ascontiguousarray(x[b].T).astype(np.float32), S),
            "wq": perm3(np.asarray(Wq)[:, cs].astype(np.float32), DC),
            "wk": perm3(np.asarray(Wk)[:, cs].astype(np.float32), DC),
            "wv": perm3(wv260, HPC * VWP),
            "wo": wo3,
            "cosT": np.concatenate([cosT64, cosT64], axis=0).astype(bf),
            "sinT": np.concatenate([sinT64, sinT64], axis=0).astype(bf),
            "rT": rT,
            "ident": ident,
            "maskT": maskT,
        })
    return in_maps


_CACHE = {}


def _get_nc(mask_key, kts, cnts, maxw):
    if mask_key not in _CACHE:
        _CACHE[mask_key] = _build_nc(kts, cnts, maxw)
    return _CACHE[mask_key]


def kernel(x, freqs_cos, freqs_sin, position_ids, bigbird_mask, Wq, Wk, Wv, Wo,
           _want_results=False, _trace=False, **trace_kwargs):
    x = np.asarray(x)
    mask = np.asarray(bigbird_mask).astype(bool)
    kts, cnts, maxw = _mask_pattern(mask)
    nc = _get_nc(mask.tobytes(), kts, cnts, maxw)
    in_maps = _host_inputs(
        x, np.asarray(freqs_cos), np.asarray(freqs_sin), np.asarray(position_ids),
        mask.astype(np.float32), kts, cnts, maxw,
        np.asarray(Wq), np.asarray(Wk), np.asarray(Wv), np.asarray(Wo),
    )
    res = bass_utils.run_bass_kernel_spmd(
        nc, in_maps, list(range(NCORES)), trace=_trace, **trace_kwargs
    )
    out = np.zeros((B, S, D), np.float32)
    for c in range(NCORES):
        out[c // HG] += res.results[c]["out"].astype(np.float32)
    if _want_results:
        return out, res
    return out


# revision 27
# speedup vs baseline: 1.1422x; 1.1422x over previous
"""BigBird attention kernel for 8 Trainium2 NeuronCores.

Sharding: data-parallel over batch (2) x tensor-parallel over heads (4 groups
of 4 heads) = 8 cores. Each core computes q/k/v projections for its head
slice, block-sparse masked attention over 128x128 supertiles derived from the
runtime mask, and a partial output projection with its Wo row-slice. The host
sums the 4 partial bf16 outputs per batch in fp32.

All matmuls run in bf16 (1 cycle/row). Scores are computed transposed
(S^T[k, q]) so the post-softmax P^T feeds attn@V directly as the stationary
operand - no per-supertile transpose matmuls. Head pairs within a 128-row
chunk run as concurrent row-tiled matmuls (tile_position (0,0)/(64,0)). The
softmax row sum rides as a ones-column appended to V; division happens on the
[q, 65] attention output (q = partition dim, native per-partition scale),
then one 128x128 transpose matmul per output chunk feeds the Wo projection.
"""

import sys

for _p in ("/opt/trn_rl_repo", "/opt/trn_rl_repo/concourse"):
    if _p not in sys.path:
        sys.path.insert(0, _p)

import numpy as np

import concourse.bacc as bacc
import concourse.bass as bass
import concourse.mybir as mybir
import concourse.tile as tile
from concourse import bass_utils

F32 = mybir.dt.float32
BF16 = mybir.dt.bfloat16

B, S, D, H = 2, 2048, 1024, 16
HD = D // H          # 64
SCALE = 1.0 / float(np.sqrt(HD))
NCORES = 8
HG = 4               # head groups (tensor-parallel)
HPC = H // HG        # heads per core = 4
DC = HPC * HD        # channels per core = 256
QT = 128             # supertile edge
NQ = S // QT         # 16
VW = HD + 1          # v columns per head incl. ones column (65)
VWP = HD + 2         # padded to 66 so bf16 head blocks stay 4B-aligned
KC = D // 128        # 8 contraction chunks
CC = DC // 128       # 2 channel chunks (2 heads each)


def _mask_pattern(mask):
    """Active 128x128 supertiles per q-tile from the runtime mask."""
    sup = mask.reshape(NQ, QT, NQ, QT).any(axis=(1, 3))  # [16,16]
    kts = [np.nonzero(sup[qi])[0].tolist() for qi in range(NQ)]
    cnts = [len(k) for k in kts]
    maxw = max(max(cnts), 1) * QT
    return kts, cnts, maxw


def _build_nc(kts, cnts, maxw):
    nc = bacc.Bacc("TRN2", target_bir_lowering=False, debug=False)

    sumw = sum(cnts) * QT
    moff = [0]
    for qi in range(NQ):
        moff.append(moff[-1] + cnts[qi] * QT)

    xT_d = nc.dram_tensor("xT", [128, KC, S], BF16, kind="ExternalInput")
    wq_d = nc.dram_tensor("wq", [128, KC, DC], BF16, kind="ExternalInput")
    wk_d = nc.dram_tensor("wk", [128, KC, DC], BF16, kind="ExternalInput")
    wv_d = nc.dram_tensor("wv", [128, KC, HPC * VWP], BF16, kind="ExternalInput")
    wo_d = nc.dram_tensor("wo", [128, CC, D], BF16, kind="ExternalInput")
    cos_d = nc.dram_tensor("cosT", [128, S], BF16, kind="ExternalInput")
    sin_d = nc.dram_tensor("sinT", [128, S], BF16, kind="ExternalInput")
    rt_d = nc.dram_tensor("rT", [128, 128], BF16, kind="ExternalInput")
    id_d = nc.dram_tensor("ident", [128, 128], BF16, kind="ExternalInput")
    mk_d = nc.dram_tensor("maskT", [128, sumw], BF16, kind="ExternalInput")
    out_d = nc.dram_tensor("out", [S, D], BF16, kind="ExternalOutput")

    with tile.TileContext(nc) as tc:
        with (
            tc.tile_pool(name="persist", bufs=1) as pp,
        ):
            # persistent sbuf tensors
            qrT = [pp.tile([128, S], BF16, tag=f"qrT{c}", name=f"qrT{c}") for c in range(CC)]
            krT = [pp.tile([128, S], BF16, tag=f"krT{c}", name=f"krT{c}") for c in range(CC)]
            v_sb = [pp.tile([128, HPC, VWP], BF16, tag=f"v{i}", name=f"v{i}") for i in range(NQ)]
            otT = pp.tile([128, CC, S], BF16, tag="otT", name="otT")
            wo_sb = pp.tile([128, CC, D], BF16, tag="wo", name="wo")
            identb = pp.tile([128, 128], BF16, tag="ident")
            mk_sb = pp.tile([128, sumw], BF16, tag="maskT", name="maskT")

            # ---------------- QKV + RoPE ----------------
            with (
                tc.tile_pool(name="qkv_in", bufs=1) as qp,
                tc.tile_pool(name="qkv_scr", bufs=4) as sp,
                tc.tile_pool(name="qkv_ps", bufs=2, space="PSUM") as psp,
                tc.tile_pool(name="qkv_rot", bufs=2, space="PSUM") as psr,
                tc.tile_pool(name="qkv_psv", bufs=2, space="PSUM") as psv,
            ):
                wq_sb = qp.tile([128, KC, DC], BF16, tag="wq", name="wq")
                wk_sb = qp.tile([128, KC, DC], BF16, tag="wk", name="wk")
                wv_sb = qp.tile([128, KC, HPC * VWP], BF16, tag="wv", name="wv")
                cosT = qp.tile([128, S], BF16, tag="cosT")
                sinT = qp.tile([128, S], BF16, tag="sinT")
                rT = qp.tile([128, 128], BF16, tag="rT")
                xT = qp.tile([128, KC, S], BF16, tag="xT", name="xT")
                # DMA order = dispatch order: first-needed first
                nc.sync.dma_start(wq_sb[:], wq_d[:, :, :])
                nc.sync.dma_start(xT[:, :, 0:512], xT_d[:, :, 0:512])
                nc.sync.dma_start(wk_sb[:], wk_d[:, :, :])
                nc.sync.dma_start(cosT[:], cos_d[:, :])
                nc.sync.dma_start(sinT[:], sin_d[:, :])
                nc.sync.dma_start(rT[:], rt_d[:, :])
                for pc in range(1, 4):
                    fs = slice(pc * 512, (pc + 1) * 512)
                    nc.sync.dma_start(xT[:, :, fs], xT_d[:, :, fs])
                nc.sync.dma_start(wv_sb[:], wv_d[:, :, :])
                nc.sync.dma_start(identb[:], id_d[:, :])
                nc.sync.dma_start(wo_sb[:], wo_d[:, :, :])
                nc.sync.dma_start(mk_sb[:], mk_d[:, :])

                # q^T / k^T with rope applied
                for pc in range(S // 512):
                    fs = slice(pc * 512, (pc + 1) * 512)
                    for cc in range(CC):
                        for w_sb, dstT in (
                            (wq_sb, qrT),
                            (wk_sb, krT),
                        ):
                            ps = psp.tile([128, 512], F32, tag="ps_qk", name="ps_qk")
                            for k in range(KC):
                                nc.tensor.matmul(
                                    ps[:],
                                    w_sb[:, k, cc * 128:(cc + 1) * 128],
                                    xT[:, k, fs],
                                    start=(k == 0),
                                    stop=(k == KC - 1),
                                )
                            raw = sp.tile([128, 512], BF16, tag="raw")
                            nc.scalar.copy(raw[:], ps[:])
                            rot = psr.tile([128, 512], F32, tag="rot")
                            nc.tensor.matmul(
                                rot[:], rT[:], raw[:], start=True, stop=True
                            )
                            u = sp.tile([128, 512], BF16, tag="u")
                            nc.vector.tensor_mul(u[:], rot[:], sinT[:, fs])
                            nc.vector.tensor_mul(dstT[cc][:, fs], raw[:], cosT[:, fs])
                            nc.vector.tensor_add(
                                dstT[cc][:, fs], dstT[cc][:, fs], u[:]
                            )

                # v natural [s, 4*66] with a ones column per head
                for pi in range(NQ):
                    ps_v = psv.tile([128, 512], F32, tag="ps_v")
                    for k in range(KC):
                        nc.tensor.matmul(
                            ps_v[:, 0:HPC * VWP],
                            xT[:, k, pi * 128:(pi + 1) * 128],
                            wv_sb[:, k, :],
                            start=(k == 0),
                            stop=(k == KC - 1),
                        )
                    nc.vector.tensor_copy(
                        v_sb[pi][:, :, :], ps_v[:, 0:HPC * VWP]
                    )
                    nc.vector.memset(v_sb[pi][:, :, HD:VW], 1.0)

            # ---------------- attention + output projection ----------------
            with (
                tc.tile_pool(name="at_p", bufs=3) as ep,
                tc.tile_pool(name="at_sc", bufs=2) as scp,
                tc.tile_pool(name="at_ob", bufs=2) as obp,
                tc.tile_pool(name="ps_s", bufs=3, space="PSUM") as pss,
                tc.tile_pool(name="ps_o", bufs=2, space="PSUM") as pso,
                tc.tile_pool(name="ps_t", bufs=1, space="PSUM") as pst,
                tc.tile_pool(name="ps_w", bufs=2, space="PSUM") as psw,
            ):
                for qi in range(NQ):
                    cnt = cnts[qi]
                    w = cnt * QT
                    qs = slice(qi * 128, (qi + 1) * 128)
                    po = pso.tile([128, HPC, 128], F32, tag="po", name="po")
                    # sub-blocks of <=4 supertiles keep scores psum in 1-bank tiles
                    subs = [(s, min(s + 4, cnt)) for s in range(0, cnt, 4)]
                    for cc in range(CC):
                        pT = [
                            ep.tile([128, maxw], BF16, tag=f"pT{hh}", name="pT")
                            for hh in range(2)
                        ]
                        for s0, s1 in subs:
                            # paired heads 2cc/2cc+1: concurrent row-tiled scores
                            ps_pair = [
                                pss.tile([128, 512], F32, tag="ps_sc", name="ps_sc")
                                for _ in range(2)
                            ]
                            for j in range(s0, s1):
                                kt = kts[qi][j]
                                for hh in range(2):
                                    ho = hh * 64
                                    nc.tensor.matmul(
                                        ps_pair[hh][:, (j - s0) * 128:(j - s0 + 1) * 128],
                                        krT[cc][ho:ho + 64, kt * 128:(kt + 1) * 128],
                                        qrT[cc][ho:ho + 64, qs],
                                        start=True,
                                        stop=True,
                                    )
                            for hh in range(2):
                                nc.scalar.activation(
                                    pT[hh][:, s0 * 128:s1 * 128],
                                    ps_pair[hh][:, 0:(s1 - s0) * 128],
                                    mybir.ActivationFunctionType.Exp,
                                    bias=0.0,
                                    scale=SCALE,
                                )
                        for hh in range(2):
                            h = 2 * cc + hh
                            pTm = ep.tile([128, maxw], BF16, tag="pTm", name="pTm")
                            nc.vector.tensor_mul(
                                pTm[:, :w], pT[hh][:, :w],
                                mk_sb[:, moff[qi]:moff[qi] + w],
                            )
                            # attn @ V' (ones column gives softmax row sums)
                            for j, kt in enumerate(kts[qi]):
                                nc.tensor.matmul(
                                    po[:, h:h + 1, 0:VW],
                                    pTm[:, j * 128:(j + 1) * 128],
                                    v_sb[kt][:, h:h + 1, 0:VW],
                                    start=(j == 0),
                                    stop=(j == cnt - 1),
                                )
                    # softmax division (per-partition scale on natural-q layout)
                    r = scp.tile([128, HPC], F32, tag="r", name="r")
                    nc.vector.reciprocal(r[:, :], po[:, :, HD:VW])
                    o_nat = scp.tile([128, DC], BF16, tag="o_nat", name="o_nat")
                    for h in range(HPC):
                        nc.vector.tensor_scalar_mul(
                            o_nat[:, h * HD:(h + 1) * HD],
                            po[:, h:h + 1, 0:HD],
                            r[:, h:h + 1],
                        )
                    # transpose O[q, c] -> otT[c, q]
                    ot_ps = pst.tile([128, 256], F32, tag="ot_ps", name="ot_ps")
                    for c2 in range(CC):
                        nc.tensor.matmul(
                            ot_ps[:, c2 * 128:(c2 + 1) * 128],
                            o_nat[:, c2 * 128:(c2 + 1) * 128],
                            identb[:],
                            start=True,
                            stop=True,
                        )
                    nc.vector.tensor_copy(otT[:, :, qs], ot_ps[:, 0:256])
                    # output projection for this q chunk
                    ob = obp.tile([128, D], BF16, tag="ob", name="ob")
                    for n2 in range(2):
                        pw = psw.tile([128, 512], F32, tag="pw", name="pw")
                        for cc2 in range(CC):
                            nc.tensor.matmul(
                                pw[:],
                                otT[:, cc2, qs],
                                wo_sb[:, cc2, n2 * 512:(n2 + 1) * 512],
                                start=(cc2 == 0),
                                stop=(cc2 == CC - 1),
                            )
                        if n2 == 0:
                            nc.vector.tensor_copy(ob[:, 0:512], pw[:])
                        else:
                            nc.scalar.copy(ob[:, 512:1024], pw[:])
                    nc.sync.dma_start(out_d[qi * 128:(qi + 1) * 128, :], ob[:])

    nc.compile()
    return nc


def _host_inputs(x, freqs_cos, freqs_sin, position_ids, mask01, kts, cnts, maxw,
                 Wq, Wk, Wv, Wo):
    """Per-core input maps."""
    import ml_dtypes
    bf = ml_dtypes.bfloat16

    in_maps = []
    r64 = np.zeros((HD, HD), np.float32)
    for i in range(HD // 2):
        r64[2 * i, 2 * i + 1] = -1.0
        r64[2 * i + 1, 2 * i] = 1.0
    r128 = np.zeros((128, 128), np.float32)
    r128[:64, :64] = r64
    r128[64:, 64:] = r64
    rT = np.ascontiguousarray(r128.T).astype(bf)
    ident = np.eye(128, dtype=np.float32).astype(bf)

    # transposed mask supertiles packed per q-tile: [k_local, off[qi] + j*128 + q_local]
    sumw = sum(cnts) * QT
    maskT = np.zeros((QT, sumw), bf)
    off = 0
    for qi in range(NQ):
        for j, kt in enumerate(kts[qi]):
            maskT[:, off + j * QT:off + (j + 1) * QT] = mask01[
                qi * QT:(qi + 1) * QT, kt * QT:(kt + 1) * QT
            ].T
        off += cnts[qi] * QT

    def perm3(a, inner):
        # [1024, inner] -> [128, 8, inner]
        return np.ascontiguousarray(
            a.reshape(KC, 128, inner).transpose(1, 0, 2)
        ).astype(bf)

    for c in range(NCORES):
        b, g = c // HG, c % HG
        pos = np.clip(position_ids[b].astype(np.int64), 0, freqs_cos.shape[0] - 1)
        cos_g = np.asarray(freqs_cos)[pos]  # [S, 32]
        sin_g = np.asarray(freqs_sin)[pos]
        cosT64 = np.repeat(cos_g.T, 2, axis=0).astype(np.float32)  # [64, S]
        sinT64 = np.repeat(sin_g.T, 2, axis=0).astype(np.float32)
        cs = slice(g * DC, (g + 1) * DC)
        wv_g = np.asarray(Wv)[:, cs].astype(np.float32)  # [D, 256]
        wv260 = np.zeros((D, HPC * VWP), np.float32)
        for h in range(HPC):
            wv260[:, h * VWP:h * VWP + HD] = wv_g[:, h * HD:(h + 1) * HD]
        wo_g = np.asarray(Wo)[cs, :].astype(np.float32)  # [256, 1024]
        wo3 = np.ascontiguousarray(
            wo_g.reshape(CC, 128, D).transpose(1, 0, 2)
        ).astype(bf)
        in_maps.append({
            "xT": perm3(np.ascontiguousarray(x[b].T).astype(np.float32), S),
            "wq": perm3(np.asarray(Wq)[:, cs].astype(np.float32), DC),
            "wk": perm3(np.asarray(Wk)[:, cs].astype(np.float32), DC),
            "wv": perm3(wv260, HPC * VWP),
            "wo": wo3,
            "cosT": np.concatenate([cosT64, cosT64], axis=0).astype(bf),
            "sinT": np.concatenate([sinT64, sinT64], axis=0).astype(bf),
            "rT": rT,
            "ident": ident,
            "maskT": maskT,
        })
    return in_maps


_CACHE = {}


def _get_nc(mask_key, kts, cnts, maxw):
    if mask_key not in _CACHE:
        _CACHE[mask_key] = _build_nc(kts, cnts, maxw)
    return _CACHE[mask_key]


def kernel(x, freqs_cos, freqs_sin, position_ids, bigbird_mask, Wq, Wk, Wv, Wo,
           _want_results=False, _trace=False, **trace_kwargs):
    x = np.asarray(x)
    mask = np.asarray(bigbird_mask).astype(bool)
    kts, cnts, maxw = _mask_pattern(mask)
    nc = _get_nc(mask.tobytes(), kts, cnts, maxw)
    in_maps = _host_inputs(
        x, np.asarray(freqs_cos), np.asarray(freqs_sin), np.asarray(position_ids),
        mask.astype(np.float32), kts, cnts, maxw,
        np.asarray(Wq), np.asarray(Wk), np.asarray(Wv), np.asarray(Wo),
    )
    res = bass_utils.run_bass_kernel_spmd(
        nc, in_maps, list(range(NCORES)), trace=_trace, **trace_kwargs
    )
    out = np.zeros((B, S, D), np.float32)
    for c in range(NCORES):
        out[c // HG] += res.results[c]["out"].astype(np.float32)
    if _want_results:
        return out, res
    return out
